# revision 2
# baseline (speedup 1.0000x reference)
"""Causal single-head attention (B=4, T=2048, D=1024, fp32) on 8 trn2 cores.

Sharding: each core takes one (batch, parity) pair: batch b = core//2,
parity p = core%2.  Within its batch, a core owns the query rows
{256*i + 2*j + p : i in 0..7, j in 0..127} -- i.e. 8 query tiles of 128
rows, where tile i holds every-other row of the global row range
[256*i, 256*(i+1)).  With a causal mask, tile i only needs keys
[0, 256*(i+1)), so the per-tile key length (2*(i+1) blocks of 128) is
identical for both parities -> one SPMD program, perfectly load-balanced,
and ~1.8x less matmul work than dense.

Per q-tile pipeline (per core):
  S = Q_tile @ K^T (PE, fp32, accumulated over 8 c-chunks into PSUM)
  bias add on the 256-col diagonal band (DVE, from the real mask input)
  row max (DVE, group-wise partial maxes pipelined behind the matmuls)
  P = exp(32*S - 32*max) (ACT, reads PSUM, writes SBUF, row-sums via
     accum_out)
  P^T per 128-block (PE transpose via identity) -> O += P^T.T @ V (PE)
  O *= 1/rowsum (DVE), DMA out.

If the mask input is NOT exactly the causal triu mask, falls back to a
dense variant of the same program (all 16 key blocks per q-tile, full
mask bias applied) which is correct for any additive {0,1} mask.
"""

import numpy as np

import concourse.bass as bass
import concourse.mybir as mybir
import concourse.tile as tile
from concourse import bacc
from concourse.bass_utils import run_bass_kernel_spmd
from concourse.masks import make_identity

B, T, D = 4, 2048, 1024
NEG = -1000000000.0
P = 128          # partitions
NCORES = 8
NQT = 8          # q-tiles of 128 rows per core
CCHUNKS = D // P  # 8 contraction chunks
STILES = T // P   # 16 key tiles per batch
F32 = mybir.dt.float32

_cache = {}


def _tile_cfg(causal: bool):
    """Per-q-tile (s_cols, bias_off, bias_cols)."""
    if causal:
        return [(256 * (i + 1), 256 * i, 256) for i in range(NQT)]
    return [(T, 0, T) for _ in range(NQT)]


def _build(causal: bool):
    cfg = _tile_cfg(causal)
    bias_cols = cfg[0][2]

    nc = bacc.Bacc("TRN2", target_bir_lowering=False, debug=False,
                   num_devices=NCORES)
    qT = nc.declare_dram_parameter("qT", [D, NQT * P], F32, isOutput=False)
    kT = nc.declare_dram_parameter("kT", [D, T], F32, isOutput=False)
    v = nc.declare_dram_parameter("v", [T, D], F32, isOutput=False)
    biasd = nc.declare_dram_parameter("bias", [NQT, P, bias_cols], F32,
                                      isOutput=False)
    out = nc.declare_dram_parameter("out", [NQT * P, D], F32, isOutput=True)

    AX = mybir.AxisListType.X
    EXP = mybir.ActivationFunctionType.Exp

    with tile.TileContext(nc) as tc:
        with (
            tc.tile_pool(name="const", bufs=1) as constp,
            tc.tile_pool(name="kv", bufs=1) as kvp,
            tc.tile_pool(name="qt", bufs=2) as qtp,
            tc.tile_pool(name="biasp", bufs=2) as biasp,
            tc.tile_pool(name="pp", bufs=2) as pp,
            tc.tile_pool(name="ptp", bufs=3) as ptp,
            tc.tile_pool(name="outp", bufs=2) as outp,
            tc.tile_pool(name="stats", bufs=4) as statp,
            tc.tile_pool(name="ps_s", bufs=1, space="PSUM") as ps_sp,
            tc.tile_pool(name="ps_t", bufs=2, space="PSUM") as ps_tp,
            tc.tile_pool(name="ps_o", bufs=1, space="PSUM") as ps_op,
        ):
            ident = constp.tile([P, P], F32)
            make_identity(nc, ident[:])

            # K^T resident: 8 c-chunk tiles [128, 2048], loaded in 512-col
            # chunks so early q-tiles don't wait on the whole tensor.
            kt_sb = []
            for c in range(CCHUNKS):
                kt_sb.append(kvp.tile([P, T], F32, tag=f"kt{c}", name=f"kt{c}"))
            for g in range(T // 512):
                for c in range(CCHUNKS):
                    nc.sync.dma_start(
                        kt_sb[c][:, g * 512:(g + 1) * 512],
                        kT[c * P:(c + 1) * P, g * 512:(g + 1) * 512])
            # V resident: 16 s-tiles [128, 1024]
            v_sb = []
            for st in range(STILES):
                v_sb.append(kvp.tile([P, D], F32, tag=f"v{st}", name=f"v{st}"))
                nc.sync.dma_start(v_sb[st][:], v[st * P:(st + 1) * P, :])

            for i in range(NQT):
                s_cols, b_off, b_cols = cfg[i]
                stiles = s_cols // P
                ngroups = (s_cols + 511) // 512

                qt_sb = qtp.tile([P, CCHUNKS * P], F32, tag="qt")
                for c in range(CCHUNKS):
                    nc.sync.dma_start(
                        qt_sb[:, c * P:(c + 1) * P],
                        qT[c * P:(c + 1) * P, i * P:(i + 1) * P])
                bias_sb = biasp.tile([P, b_cols], F32, tag="bias")
                nc.sync.dma_start(bias_sb[:], biasd[i])

                ps = ps_sp.tile([P, s_cols], F32, tag="s")
                pmax = statp.tile([P, ngroups], F32, tag="pmax")
                for g in range(ngroups):
                    g0 = g * 512
                    gw = min(512, s_cols - g0)
                    for c in range(CCHUNKS):
                        nc.tensor.matmul(
                            ps[:, g0:g0 + gw],
                            qt_sb[:, c * P:(c + 1) * P],
                            kt_sb[c][:, g0:g0 + gw],
                            start=(c == 0), stop=(c == CCHUNKS - 1))
                    # additive mask bias overlapping this group
                    lo = max(g0, b_off)
                    hi = min(g0 + gw, b_off + b_cols)
                    if lo < hi:
                        nc.vector.tensor_add(
                            ps[:, lo:hi], ps[:, lo:hi],
                            bias_sb[:, lo - b_off:hi - b_off])
                    nc.vector.reduce_max(pmax[:, g:g + 1], ps[:, g0:g0 + gw],
                                         axis=AX)
                m = statp.tile([P, 1], F32, tag="m")
                nc.vector.reduce_max(m[:], pmax[:, :ngroups], axis=AX)
                negm32 = statp.tile([P, 1], F32, tag="negm32")
                nc.scalar.mul(negm32[:], m[:], -32.0)

                p_sb = pp.tile([P, s_cols], F32, tag="p")
                gsum = statp.tile([P, ngroups], F32, tag="gsum")
                for g in range(ngroups):
                    g0 = g * 512
                    gw = min(512, s_cols - g0)
                    nc.scalar.activation(
                        p_sb[:, g0:g0 + gw], ps[:, g0:g0 + gw], EXP,
                        bias=negm32[:], scale=32.0,
                        accum_out=gsum[:, g:g + 1])
                rsum = statp.tile([P, 1], F32, tag="rsum")
                nc.vector.reduce_sum(rsum[:], gsum[:, :ngroups], axis=AX)
                rinv = statp.tile([P, 1], F32, tag="rinv")
                nc.vector.reciprocal(rinv[:], rsum[:])

                ps_o = ps_op.tile([P, D], F32, tag="o")
                for st in range(stiles):
                    ps_t = ps_tp.tile([P, P], F32, tag="t")
                    nc.tensor.transpose(ps_t[:], p_sb[:, st * P:(st + 1) * P],
                                        ident[:])
                    pt_sb = ptp.tile([P, P], F32, tag="pt")
                    nc.vector.tensor_copy(pt_sb[:], ps_t[:])
                    for dh in range(2):
                        nc.tensor.matmul(
                            ps_o[:, dh * 512:(dh + 1) * 512],
                            pt_sb[:],
                            v_sb[st][:, dh * 512:(dh + 1) * 512],
                            start=(st == 0), stop=(st == stiles - 1))
                o_sb = outp.tile([P, D], F32, tag="o_sb")
                nc.vector.tensor_scalar_mul(o_sb[:], ps_o[:], rinv[:])
                nc.sync.dma_start(out[i * P:(i + 1) * P, :], o_sb[:])

    nc.compile()
    return nc


def _rows(causal: bool, p: int) -> np.ndarray:
    if causal:
        return np.concatenate(
            [256 * i + 2 * np.arange(P) + p for i in range(NQT)])
    return p * (NQT * P) + np.arange(NQT * P)


def _get(causal: bool):
    if causal not in _cache:
        _cache[causal] = _build(causal)
    return _cache[causal]


def kernel(query, key, value, mask):
    query = np.asarray(query, dtype=np.float32)
    key = np.asarray(key, dtype=np.float32)
    value = np.asarray(value, dtype=np.float32)
    mask = np.asarray(mask, dtype=np.float32)

    causal = bool(
        np.array_equal(mask, np.triu(np.ones((T, T), np.float32), k=1)))
    nc = _get(causal)
    cfg = _tile_cfg(causal)
    # bias folded pre-scale: 32*(S + mask*NEG/32) == 32*S + mask*NEG exactly
    mask_scaled = mask * np.float32(NEG / 32.0)

    kTs = [np.ascontiguousarray(key[b].T) for b in range(B)]
    in_maps = []
    rows_by_core = []
    for c in range(NCORES):
        b, p = c // 2, c % 2
        rows = _rows(causal, p)
        rows_by_core.append((b, rows))
        qT_c = np.ascontiguousarray(query[b][rows].T)
        bias_c = np.stack([
            mask_scaled[rows[i * P:(i + 1) * P], boff:boff + bcols]
            for i, (_, boff, bcols) in enumerate(cfg)])
        in_maps.append({
            "qT": qT_c,
            "kT": kTs[b],
            "v": np.ascontiguousarray(value[b]),
            "bias": np.ascontiguousarray(bias_c),
        })

    res = run_bass_kernel_spmd(nc, in_maps, core_ids=list(range(NCORES)))

    outp = np.empty((B, T, D), dtype=np.float32)
    for c in range(NCORES):
        b, rows = rows_by_core[c]
        outp[b][rows] = res.results[c]["out"]
    return outp


# revision 10
# speedup vs baseline: 1.8326x; 1.8326x over previous
"""Causal single-head attention (B=4, T=2048, D=1024, fp32) on 8 trn2 cores.

Sharding: each core takes one (batch, parity) pair: batch b = core//2,
parity p = core%2.  Within its batch, a core owns the query rows
{256*i + 2*j + p : i in 0..7, j in 0..127} -- i.e. 8 query tiles of 128
rows, where tile i holds every-other row of the global row range
[256*i, 256*(i+1)).  With a causal mask, tile i only needs keys
[0, 256*(i+1)), so the per-tile key length (2*(i+1) blocks of 128) is
identical for both parities -> one SPMD program, perfectly load-balanced,
and ~1.8x less matmul work than dense.

Per q-tile pipeline (per core):
  S = Q_tile @ K^T (PE, fp32, accumulated over 8 c-chunks into PSUM)
  bias add on the 256-col diagonal band (DVE, from the real mask input)
  row max (DVE, group-wise partial maxes pipelined behind the matmuls)
  P = exp(32*S - 32*max) (ACT, reads PSUM, writes SBUF, row-sums via
     accum_out)
  P^T per 128-block (PE transpose via identity) -> O += P^T.T @ V (PE)
  O *= 1/rowsum (DVE), DMA out.

If the mask input is NOT exactly the causal triu mask, falls back to a
dense variant of the same program (all 16 key blocks per q-tile, full
mask bias applied) which is correct for any additive {0,1} mask.
"""

import numpy as np

import concourse.bass as bass
import concourse.mybir as mybir
import concourse.tile as tile
from concourse import bacc
from concourse.bass_utils import run_bass_kernel_spmd
from concourse.masks import make_identity

B, T, D = 4, 2048, 1024
NEG = -1000000000.0
P = 128          # partitions
NCORES = 8
NQT = 8          # q-tiles of 128 rows per core
CCHUNKS = D // P  # 8 contraction chunks
STILES = T // P   # 16 key tiles per batch
F32 = mybir.dt.float32
F16 = mybir.dt.float16

# AV (P @ V) operand dtype: fp16 runs the PE at 1 cycle/row vs fp32's 4.
# P in [0,1] and V ~ N(0,1) both fit fp16 with ~2^-11 relative rounding.
import os
AV_DT = F32 if os.environ.get("KERNEL_AV_F32", "0") == "1" else F16
_cache = {}


def _tile_cfg(causal: bool):
    """Per-q-tile (s_cols, bias_off, bias_cols)."""
    if causal:
        return [(256 * (i + 1), 256 * i, 256) for i in range(NQT)]
    return [(T, 0, T) for _ in range(NQT)]


def _build(causal: bool):
    cfg = _tile_cfg(causal)
    bias_cols = cfg[0][2]

    nc = bacc.Bacc("TRN2", target_bir_lowering=False, debug=False,
                   num_devices=NCORES)
    qT = nc.declare_dram_parameter("qT", [D, NQT * P], F32, isOutput=False)
    kT = nc.declare_dram_parameter("kT", [D, T], F32, isOutput=False)
    v = nc.declare_dram_parameter("v", [T, D], AV_DT, isOutput=False)
    biasd = nc.declare_dram_parameter("bias", [NQT, P, bias_cols], F32,
                                      isOutput=False)
    out = nc.declare_dram_parameter("out", [NQT * P, D], F32, isOutput=True)

    AX = mybir.AxisListType.X
    EXP = mybir.ActivationFunctionType.Exp

    with tile.TileContext(nc) as tc:
        with (
            tc.tile_pool(name="const", bufs=1) as constp,
            tc.tile_pool(name="kv", bufs=1) as kvp,
            tc.tile_pool(name="qt", bufs=2) as qtp,
            tc.tile_pool(name="biasp", bufs=2) as biasp,
            tc.tile_pool(name="pp", bufs=2) as pp,
            tc.tile_pool(name="ssb", bufs=2) as ssbp,
            tc.tile_pool(name="ptp", bufs=3) as ptp,
            tc.tile_pool(name="outp", bufs=2) as outp,
            tc.tile_pool(name="stats", bufs=4) as statp,
            tc.tile_pool(name="ps_s", bufs=4, space="PSUM") as ps_sp,
            tc.tile_pool(name="ps_t", bufs=2, space="PSUM") as ps_tp,
            tc.tile_pool(name="ps_o", bufs=1, space="PSUM") as ps_op,
        ):
            ident = constp.tile([P, P], AV_DT)
            make_identity(nc, ident[:])

            # K^T / V stay SBUF-resident; their loads are emitted inside the
            # q-tile loop in consumption order so q-tile 0's operands aren't
            # queued behind 16MB of K/V DMA.
            kt_sb = []
            for c in range(CCHUNKS):
                kt_sb.append(kvp.tile([P, T], F32, tag=f"kt{c}", name=f"kt{c}"))
            v_sb = []
            for st in range(STILES):
                v_sb.append(kvp.tile([P, D], AV_DT, tag=f"v{st}", name=f"v{st}"))
            warm = constp.tile([P, 512], F32, name="warm")
            nc.gpsimd.memset(warm[:], 0.0)
            for w in range(12):
                ps_w = ps_sp.tile([P, 512], F32, tag="s", name="ps_w")
                nc.tensor.matmul(ps_w[:], warm[:, :P], warm[:],
                                 start=True, stop=True)

            kt_loaded = 0  # next 512-col chunk of kT to load
            v_loaded = 0   # next s-tile of V to load
            max_scols = max(sc for sc, _, _ in cfg)

            state = {}  # q-tile -> tensors produced by stage A

            def stage_a(i):
                """DMAs + QK matmuls into per-group PSUM, copy to SBUF S,
                mask bias add, row-max stats."""
                s_cols, b_off, b_cols = cfg[i]
                ngroups = (s_cols + 511) // 512

                qt_sb = qtp.tile([P, CCHUNKS * P], F32, tag="qt", name="qt_sb")
                for c in range(CCHUNKS):
                    nc.sync.dma_start(
                        qt_sb[:, c * P:(c + 1) * P],
                        qT[c * P:(c + 1) * P, i * P:(i + 1) * P])
                # kT column chunks first used by this q-tile (plus one chunk
                # of lookahead), then V s-tiles this q-tile newly needs.
                nonlocal kt_loaded, v_loaded
                want_kt = min((min(s_cols + 512, max_scols) + 511) // 512,
                              T // 512)
                while kt_loaded < want_kt:
                    g = kt_loaded
                    for c in range(CCHUNKS):
                        nc.sync.dma_start(
                            kt_sb[c][:, g * 512:(g + 1) * 512],
                            kT[c * P:(c + 1) * P, g * 512:(g + 1) * 512])
                    kt_loaded += 1
                want_v = min(s_cols // P + 2, STILES) if causal else STILES
                while v_loaded < want_v:
                    st = v_loaded
                    nc.sync.dma_start(v_sb[st][:], v[st * P:(st + 1) * P, :])
                    v_loaded += 1
                bias_sb = biasp.tile([P, b_cols], F32, tag="bias",
                                     name="bias_sb")
                nc.sync.dma_start(bias_sb[:], biasd[i])

                s_sb = ssbp.tile([P, s_cols], F32, tag="s_sb", name="s_sb")
                pmax = statp.tile([P, ngroups], F32, tag="pmax", name="pmax")
                for g in range(ngroups):
                    g0 = g * 512
                    gw = min(512, s_cols - g0)
                    ps = ps_sp.tile([P, 512], F32, tag="s", name="ps_g")
                    for c in range(CCHUNKS):
                        nc.tensor.matmul(
                            ps[:, :gw],
                            qt_sb[:, c * P:(c + 1) * P],
                            kt_sb[c][:, g0:g0 + gw],
                            start=(c == 0), stop=(c == CCHUNKS - 1))
                    # PSUM -> SBUF: plain copy outside the mask band (ACT),
                    # fused bias-add inside it (DVE).
                    lo = max(g0, b_off)
                    hi = min(g0 + gw, b_off + b_cols)
                    if lo < hi:
                        if lo > g0:
                            nc.scalar.copy(s_sb[:, g0:lo], ps[:, :lo - g0])
                        nc.vector.tensor_add(
                            s_sb[:, lo:hi], ps[:, lo - g0:hi - g0],
                            bias_sb[:, lo - b_off:hi - b_off])
                        if hi < g0 + gw:
                            nc.scalar.copy(s_sb[:, hi:g0 + gw],
                                           ps[:, hi - g0:gw])
                    else:
                        nc.scalar.copy(s_sb[:, g0:g0 + gw], ps[:, :gw])
                    nc.vector.reduce_max(pmax[:, g:g + 1], s_sb[:, g0:g0 + gw],
                                         axis=AX)
                negm = statp.tile([P, 1], F32, tag="negm", name="negm")
                nc.vector.reduce_max(negm[:], pmax[:, :ngroups], axis=AX,
                                     negate=True)
                negm32 = statp.tile([P, 1], F32, tag="negm32", name="negm32")
                nc.vector.tensor_scalar_mul(negm32[:], negm[:], 32.0)
                state[i] = (s_sb, negm32)

            def stage_b(i):
                """exp + row-sum, P^T transposes, AV accumulation, 1/sum
                scale, output DMA."""
                s_cols, _, _ = cfg[i]
                stiles = s_cols // P
                ngroups = (s_cols + 511) // 512
                s_sb, negm32 = state.pop(i)

                p_sb = pp.tile([P, s_cols], AV_DT, tag="p", name="p_sb")
                gsum = statp.tile([P, ngroups], F32, tag="gsum", name="gsum")
                for g in range(ngroups):
                    g0 = g * 512
                    gw = min(512, s_cols - g0)
                    nc.scalar.activation(
                        p_sb[:, g0:g0 + gw], s_sb[:, g0:g0 + gw], EXP,
                        bias=negm32[:], scale=32.0,
                        accum_out=gsum[:, g:g + 1])
                rsum = statp.tile([P, 1], F32, tag="rsum", name="rsum")
                nc.vector.reduce_sum(rsum[:], gsum[:, :ngroups], axis=AX)
                rinv = statp.tile([P, 1], F32, tag="rinv", name="rinv")
                nc.vector.reciprocal(rinv[:], rsum[:])

                ps_o = ps_op.tile([P, D], F32, tag="o", name="ps_o")
                for st in range(stiles):
                    ps_t = ps_tp.tile([P, P], AV_DT, tag="t", name="ps_t")
                    nc.tensor.transpose(ps_t[:], p_sb[:, st * P:(st + 1) * P],
                                        ident[:])
                    pt_sb = ptp.tile([P, P], AV_DT, tag="pt", name="pt_sb")
                    nc.vector.tensor_copy(pt_sb[:], ps_t[:])
                    for dh in range(2):
                        nc.tensor.matmul(
                            ps_o[:, dh * 512:(dh + 1) * 512],
                            pt_sb[:],
                            v_sb[st][:, dh * 512:(dh + 1) * 512],
                            start=(st == 0), stop=(st == stiles - 1))
                o_sb = outp.tile([P, D], F32, tag="o_sb", name="o_sb")
                nc.vector.tensor_scalar_mul(o_sb[:], ps_o[:], rinv[:])
                nc.sync.dma_start(out[i * P:(i + 1) * P, :], o_sb[:])

            # Software pipeline: QK of one tile runs (on PE) while the
            # previous tile does softmax/exp on ACT/DVE, so PE never waits
            # on the softmax.  Tile 2 is moved last so the un-overlapped
            # final B stage is a small one (6 key blocks instead of 16).
            order = [0, 1, 3, 4, 5, 6, 7, 2]
            for idx in range(len(order) + 1):
                if idx < len(order):
                    stage_a(order[idx])
                if idx > 0:
                    stage_b(order[idx - 1])

    nc.compile()
    return nc


def _rows(causal: bool, p: int) -> np.ndarray:
    if causal:
        return np.concatenate(
            [256 * i + 2 * np.arange(P) + p for i in range(NQT)])
    return p * (NQT * P) + np.arange(NQT * P)


def _get(causal: bool):
    if causal not in _cache:
        _cache[causal] = _build(causal)
    return _cache[causal]


def kernel(query, key, value, mask):
    query = np.asarray(query, dtype=np.float32)
    key = np.asarray(key, dtype=np.float32)
    value = np.asarray(value, dtype=np.float32)
    mask = np.asarray(mask, dtype=np.float32)

    causal = bool(
        np.array_equal(mask, np.triu(np.ones((T, T), np.float32), k=1)))
    nc = _get(causal)
    cfg = _tile_cfg(causal)
    # bias folded pre-scale: 32*(S + mask*NEG/32) == 32*S + mask*NEG exactly
    mask_scaled = mask * np.float32(NEG / 32.0)

    kTs = [np.ascontiguousarray(key[b].T) for b in range(B)]
    in_maps = []
    rows_by_core = []
    for c in range(NCORES):
        b, p = c // 2, c % 2
        rows = _rows(causal, p)
        rows_by_core.append((b, rows))
        qT_c = np.ascontiguousarray(query[b][rows].T)
        bias_c = np.stack([
            mask_scaled[rows[i * P:(i + 1) * P], boff:boff + bcols]
            for i, (_, boff, bcols) in enumerate(cfg)])
        in_maps.append({
            "qT": qT_c,
            "kT": kTs[b],
            "v": np.ascontiguousarray(value[b]).astype(
                np.float16 if AV_DT == F16 else np.float32),
            "bias": np.ascontiguousarray(bias_c),
        })

    res = run_bass_kernel_spmd(nc, in_maps, core_ids=list(range(NCORES)))

    outp = np.empty((B, T, D), dtype=np.float32)
    for c in range(NCORES):
        b, rows = rows_by_core[c]
        outp[b][rows] = res.results[c]["out"]
    return outp
